# revision 5
# baseline (speedup 1.0000x reference)
"""Trainium2 Bass kernel for nn_AnomalyGenerator.

Math (reference):
  g[h,w,k]   = pi[k] * exp(-(zx^2 - 2 rho zx zy + zy^2) / (2 (1 - rho^2)))
  s[h,w]     = sum_k g
  m          = max_{h,w} s           (global over the full image)
  anomaly    = s * scale / m         (== 1 - mask)
  noise_c    = (sum_k g * color[c,k]) * scale / m
  imageA_c   = imageN_c * (1 - anomaly) + noise_c

Key restructure: the division by the global max commutes with the k-reduction,
so pass 1 computes s and raw noise nr_c per pixel (never materializing the
[H,W,K] tensor), one scalar AllReduce(max), then a light elementwise pass 2.

The exp argument separates as  arg = U[k,w]*zy[h,k] + bias[h,k] + D[k,w]  with
  q = 1/(2(1-rho^2)),  zx=(x-mux)/sx, zy=(y-muy)/sy
  U = 2 q rho zx,  D = -q zx^2,  bias = -q zy^2 + ln pi
One custom DVE op (affine_then_add) assembles arg in f32 per row-pair
(partition dim = 2 rows x 64 k), ScalarE does exp -> bf16, and TensorE reduces
over k via a block-diagonal [128,8] matmul producing (s, n0, n1, n2) for the
two rows at once.

Sharding: H=1024 rows split across 8 cores (128 rows each); only the scalar
max needs a collective.
"""

import numpy as np

H = 1024
W = 1024
K = 64
C = 3
NCORES = 8
SH = H // NCORES  # 128 rows per core
NPAIR = SH // 2  # 64 row-pairs per core

_CACHE = {}


def _build_nc():
    import concourse.bacc as bacc
    import concourse.mybir as mybir
    import concourse.tile as tile

    fp32 = mybir.dt.float32
    bf16 = mybir.dt.bfloat16
    AF = mybir.ActivationFunctionType
    ALU = mybir.AluOpType
    AX = mybir.AxisListType

    nc = bacc.Bacc(
        "TRN2",
        target_bir_lowering=False,
        debug=False,
        enable_asserts=False,
        num_devices=NCORES,
    )

    img_d = nc.dram_tensor("imgN", [C, SH, W], fp32, kind="ExternalInput").ap()
    U2_d = nc.dram_tensor("U2", [128, W], fp32, kind="ExternalInput").ap()
    D2_d = nc.dram_tensor("D2", [128, W], fp32, kind="ExternalInput").ap()
    zyT_d = nc.dram_tensor("zyT", [128, NPAIR], fp32, kind="ExternalInput").ap()
    biasT_d = nc.dram_tensor("biasT", [128, NPAIR], fp32, kind="ExternalInput").ap()
    caug_d = nc.dram_tensor("caug", [128, 8], bf16, kind="ExternalInput").ap()
    # [1,2] = [scale, -scale]; multiplied by 1/gmax on device.
    sgn_d = nc.dram_tensor("sgn", [1, 2], fp32, kind="ExternalInput").ap()

    outA_d = nc.dram_tensor("outA", [C, SH, W], fp32, kind="ExternalOutput").ap()
    outM_d = nc.dram_tensor("outM", [SH, W], fp32, kind="ExternalOutput").ap()

    with tile.TileContext(nc) as tc:
        with (
            tc.tile_pool(name="const", bufs=1) as cpool,
            tc.tile_pool(name="arg", bufs=3) as apool,
            tc.tile_pool(name="g", bufs=3) as gpool,
            tc.tile_pool(name="stage", bufs=2) as spool,
            tc.tile_pool(name="big", bufs=1) as bpool,
            tc.tile_pool(name="v", bufs=1) as vpool,
            tc.tile_pool(name="oa", bufs=2) as opool,
            tc.tile_pool(name="psum", bufs=3, space="PSUM") as ppool,
            tc.tile_pool(name="psb", bufs=1, space="PSUM") as pbpool,
            tc.tile_pool(name="dram", bufs=1, space="DRAM") as dpool,
        ):
            U2 = cpool.tile([128, W], fp32, tag="U2")
            nc.sync.dma_start(U2[:], U2_d[:])
            D2 = cpool.tile([128, W], fp32, tag="D2")
            nc.sync.dma_start(D2[:], D2_d[:])
            zyT = cpool.tile([128, NPAIR], fp32, tag="zyT")
            nc.sync.dma_start(zyT[:], zyT_d[:])
            biasT = cpool.tile([128, NPAIR], fp32, tag="biasT")
            nc.sync.dma_start(biasT[:], biasT_d[:])
            caug = cpool.tile([128, 8], bf16, tag="caug")
            nc.sync.dma_start(caug[:], caug_d[:])
            sgn = cpool.tile([1, 2], fp32, tag="sgn")
            nc.sync.dma_start(sgn[:], sgn_d[:])

            # imageN prefetch (used only in pass 2): SBUF layout [h, c, w]
            img = bpool.tile([128, C, W], fp32, tag="img")
            nc.sync.dma_start(img[:], img_d.rearrange("c h w -> h c w"))

            # pass-1 accumulator: [h, ch, w] with ch = (s, n0, n1, n2)
            out4 = bpool.tile([128, 4, W], fp32, tag="out4")

            # ---------------- pass 1: 64 row-pairs, 4 per PSUM group --------
            # Each pair's [8, W] matmul output lands at partition offset
            # 32*jj of a [128, W] PSUM group tile (legal tile_position col
            # offsets are multiples of 32 for <=32-row outputs). One ScalarE
            # copy evacuates the group; small DMAs remap partitions into the
            # [h, ch, w] accumulator.
            for g in range(NPAIR // 4):
                Pg = ppool.tile([128, W], fp32, tag="P")
                for jj in range(4):
                    j = 4 * g + jj
                    arg = apool.tile([128, W], fp32, tag="arg")
                    nc.vector.affine_then_add(
                        arg[:], U2[:], D2[:],
                        scale=zyT[:, j : j + 1], bias=biasT[:, j : j + 1],
                    )
                    G = gpool.tile([128, W], bf16, tag="G")
                    nc.scalar.activation(G[:], arg[:], AF.Exp)
                    p0 = 32 * jj
                    nc.tensor.matmul(
                        Pg[p0 : p0 + 8, 0:512], caug[:], G[:, 0:512],
                        tile_position=(0, p0),
                    )
                    nc.tensor.matmul(
                        Pg[p0 : p0 + 8, 512:1024], caug[:], G[:, 512:1024],
                        tile_position=(0, p0),
                    )
                Sg = spool.tile([128, W], fp32, tag="S")
                nc.scalar.activation(Sg[:], Pg[:], AF.Copy)
                for jj in range(4):
                    h0 = 8 * g + 2 * jj
                    # stage rows 32jj..32jj+7 = (s,n0,n1,n2) x (h0, h0+1)
                    nc.sync.dma_start(
                        out4[h0 : h0 + 2, :, :], Sg[32 * jj : 32 * jj + 8, :]
                    )

            # ---------------- local max over s ----------------
            pmax = cpool.tile([128, 1], fp32, tag="pmax")
            nc.vector.reduce_max(pmax[:], out4[:, 0, :], axis=AX.X)
            pmax_row = cpool.tile([1, 128], fp32, tag="pmax_row")
            nc.sync.dma_start(pmax_row[:], pmax[:])
            m8 = cpool.tile([1, 8], fp32, tag="m8")
            nc.vector.reduce_max(m8[:, 0:1], pmax_row[:], axis=AX.X)
            ones8 = cpool.tile([1, 8], fp32, tag="ones8")
            nc.vector.memset(ones8[:], 1.0)
            mbc = cpool.tile([1, 8], fp32, tag="mbc")
            nc.vector.tensor_scalar_mul(mbc[:], ones8[:], m8[:, 0:1])

            # ---------------- AllReduce(max) ----------------
            cc_in = dpool.tile([1, 8], fp32, tag="cc_in")
            cc_out = dpool.tile([1, 8], fp32, tag="cc_out")
            nc.sync.dma_start(cc_in[:], mbc[:])
            nc.gpsimd.collective_compute(
                "AllReduce",
                ALU.max,
                replica_groups=[list(range(NCORES))],
                ins=[cc_in.opt()],
                outs=[cc_out.opt()],
            )
            gmax = cpool.tile([1, 1], fp32, tag="gmax")
            nc.sync.dma_start(gmax[:], cc_out[0:1, 0:1])

            # V_c = imageN_c * s - nr_c  (independent of gmax: overlaps the
            # collective). Post-collective: imageA_c = imageN_c - inv*V_c.
            V = []
            for c in range(C):
                vt = vpool.tile([128, W], fp32, tag=f"V{c}")
                nc.vector.tensor_mul(vt[:], img[:, c, :], out4[:, 0, :])
                nc.vector.tensor_sub(vt[:], vt[:], out4[:, 1 + c, :])
                V.append(vt)

            # inv = +/- scale/gmax, broadcast to all 128 partitions via PE
            rec = cpool.tile([1, 1], fp32, tag="rec")
            nc.vector.reciprocal(rec[:], gmax[:])
            invpair = cpool.tile([1, 2], fp32, tag="invpair")
            nc.vector.tensor_scalar_mul(invpair[:], sgn[:], rec[:])
            ones_r = cpool.tile([1, 128], fp32, tag="ones_r")
            nc.vector.memset(ones_r[:], 1.0)
            ps_inv = pbpool.tile([128, 2], fp32, tag="ps_inv")
            nc.tensor.matmul(ps_inv[:], ones_r[:], invpair[:])
            inv2 = cpool.tile([128, 2], fp32, tag="inv2")
            nc.vector.tensor_copy(inv2[:], ps_inv[:])

            # ---------------- pass 2 ----------------
            an = opool.tile([128, W], fp32, tag="an")
            nc.vector.tensor_scalar_mul(an[:], out4[:, 0, :], inv2[:, 0:1])
            nc.sync.dma_start(outM_d[:], an[:])
            for c in range(C):
                oA = opool.tile([128, W], fp32, tag="oA")
                # (V_c * -inv) + imageN_c
                nc.vector.scalar_tensor_tensor(
                    oA[:],
                    in0=V[c][:],
                    scalar=inv2[:, 1:2],
                    in1=img[:, c, :],
                    op0=ALU.mult,
                    op1=ALU.add,
                )
                nc.sync.dma_start(outA_d[c], oA[:])

    nc.finalize()
    return nc


def _precompute(color_params, mu_x_params, mu_y_params, sigma_x_params,
                sigma_y_params, rho_params, pi_params, scale_params):
    """CPU parameter transforms (O(K*(H+W)) work) -> per-core input arrays."""
    import ml_dtypes

    f64 = np.float64
    cp = color_params.reshape(C, K).astype(f64)
    mux = np.tanh(mu_x_params.reshape(K).astype(f64))
    muy = np.tanh(mu_y_params.reshape(K).astype(f64))
    sx = np.logaddexp(0.0, sigma_x_params.reshape(K).astype(f64)) + 0.01
    sy = np.logaddexp(0.0, sigma_y_params.reshape(K).astype(f64)) + 0.01
    rho = np.tanh(rho_params.reshape(K).astype(f64)) * 0.99
    pi_p = pi_params.reshape(K).astype(f64)
    lnpi = pi_p - (np.log(np.sum(np.exp(pi_p - pi_p.max()))) + pi_p.max())
    scale = 1.0 / (1.0 + np.exp(-float(scale_params.reshape(()))))
    color = np.tanh(cp)

    q = 1.0 / (2.0 * (1.0 - rho * rho))  # [K]

    # mimic the reference's f32 grid
    x = (np.arange(W, dtype=np.float32) / (W - 1) * 2.0 - 1.0).astype(f64)
    y = (np.arange(H, dtype=np.float32) / (H - 1) * 2.0 - 1.0).astype(f64)

    zx = (x[None, :] - mux[:, None]) / sx[:, None]  # [K, W]
    U = 2.0 * (q * rho)[:, None] * zx  # [K, W]
    D = -q[:, None] * zx * zx  # [K, W]
    zy = (y[None, :] - muy[:, None]) / sy[:, None]  # [K, H]
    e = -q[:, None] * zy * zy + lnpi[:, None]  # [K, H]

    U2 = np.concatenate([U, U], axis=0).astype(np.float32)  # [128, W]
    D2 = np.concatenate([D, D], axis=0).astype(np.float32)

    caug = np.zeros((128, 8), np.float32)
    caug[0:K, 0] = 1.0
    caug[K:128, 4] = 1.0
    for c in range(C):
        caug[0:K, 1 + c] = color[c]
        caug[K:128, 5 + c] = color[c]
    caug = caug.astype(ml_dtypes.bfloat16)

    sgn = np.array([[scale, -scale]], np.float32)

    zyTs, biasTs = [], []
    for i in range(NCORES):
        zc = zy[:, i * SH : (i + 1) * SH]  # [K, 128]
        ec = e[:, i * SH : (i + 1) * SH]
        zyTs.append(
            np.concatenate([zc[:, 0::2], zc[:, 1::2]], axis=0).astype(np.float32)
        )
        biasTs.append(
            np.concatenate([ec[:, 0::2], ec[:, 1::2]], axis=0).astype(np.float32)
        )
    return U2, D2, caug, sgn, zyTs, biasTs


def _run(inputs, trace=False):
    from concourse.bass_utils import run_bass_kernel_spmd

    if "nc" not in _CACHE:
        _CACHE["nc"] = _build_nc()
    nc = _CACHE["nc"]

    imageN = np.ascontiguousarray(np.asarray(inputs["imageN"], np.float32))
    U2, D2, caug, sgn, zyTs, biasTs = _precompute(
        inputs["color_params"], inputs["mu_x_params"], inputs["mu_y_params"],
        inputs["sigma_x_params"], inputs["sigma_y_params"], inputs["rho_params"],
        inputs["pi_params"], inputs["scale_params"],
    )

    in_maps = []
    for i in range(NCORES):
        in_maps.append(
            {
                "imgN": np.ascontiguousarray(imageN[0, :, i * SH : (i + 1) * SH, :]),
                "U2": U2,
                "D2": D2,
                "zyT": zyTs[i],
                "biasT": biasTs[i],
                "caug": caug,
                "sgn": sgn,
            }
        )

    res = run_bass_kernel_spmd(nc, in_maps, core_ids=list(range(NCORES)), trace=trace)

    imageA = np.empty((1, C, H, W), np.float32)
    anomaly = np.empty((1, 1, H, W), np.float32)
    for i in range(NCORES):
        imageA[0, :, i * SH : (i + 1) * SH, :] = res.results[i]["outA"]
        anomaly[0, 0, i * SH : (i + 1) * SH, :] = res.results[i]["outM"]
    return (imageA, anomaly), res


def kernel(**inputs):
    out, _ = _run(inputs, trace=False)
    return out


# revision 10
# speedup vs baseline: 1.1057x; 1.1057x over previous
"""Trainium2 Bass kernel for nn_AnomalyGenerator.

Math (reference):
  g[h,w,k]   = pi[k] * exp(-(zx^2 - 2 rho zx zy + zy^2) / (2 (1 - rho^2)))
  s[h,w]     = sum_k g
  m          = max_{h,w} s           (global over the full image)
  anomaly    = s * scale / m         (== 1 - mask)
  noise_c    = (sum_k g * color[c,k]) * scale / m
  imageA_c   = imageN_c * (1 - anomaly) + noise_c

Key restructure: the division by the global max commutes with the k-reduction,
so pass 1 computes s and raw noise nr_c per pixel (never materializing the
[H,W,K] tensor), a scalar AllReduce(max), then a light elementwise pass 2.

The exp argument separates as  arg = U[k,w]*zy[h,k] + bias[h,k] + D[k,w]  with
  q = 1/(2(1-rho^2)),  zx=(x-mux)/sx, zy=(y-muy)/sy
  U = 2 q rho zx,  D = -q zx^2,  bias = -q zy^2 + ln pi
One custom DVE op (affine_then_add) assembles arg in f32 per row-pair
(partition dim = 2 rows x 64 k) -- f32 assembly is required: U*zy and D can
each be ~1e3 with a small difference, so a factored exp overflows f32.
ScalarE exps 4 row-pairs per instruction -> bf16, TensorE reduces over k via a
block-diagonal [128,8] matmul (s, n0, n1, n2 for two rows at once), 4 pairs
per [128,W] PSUM group at partition offsets 0/32/64/96, one ScalarE copy
evacuates the group, tiny DMAs remap partitions into the [h, ch, w] layout.

The global max is pipelined: each half of the rows does its own
AllReduce(max) as soon as it finishes, so only the second (tiny) collective's
latency is exposed; a dummy warmup collective runs at kernel start.

Sharding: H=1024 rows split across 8 cores (128 rows each).
"""

import numpy as np

H = 1024
W = 1024
K = 64
C = 3
NCORES = 8
SH = H // NCORES  # 128 rows per core
NPAIR = SH // 2  # 64 row-pairs per core
NGRP = NPAIR // 4  # 16 groups of 4 pairs

_CACHE = {}


def _build_nc():
    import concourse.bacc as bacc
    import concourse.mybir as mybir
    import concourse.tile as tile

    fp32 = mybir.dt.float32
    bf16 = mybir.dt.bfloat16
    AF = mybir.ActivationFunctionType
    ALU = mybir.AluOpType
    AX = mybir.AxisListType
    RG = [list(range(NCORES))]

    nc = bacc.Bacc(
        "TRN2",
        target_bir_lowering=False,
        debug=False,
        enable_asserts=False,
        num_devices=NCORES,
    )

    img_d = nc.dram_tensor("imgN", [C, SH, W], fp32, kind="ExternalInput").ap()
    U2_d = nc.dram_tensor("U2", [128, W], fp32, kind="ExternalInput").ap()
    D2_d = nc.dram_tensor("D2", [128, W], fp32, kind="ExternalInput").ap()
    zyT_d = nc.dram_tensor("zyT", [128, NPAIR], fp32, kind="ExternalInput").ap()
    biasT_d = nc.dram_tensor("biasT", [128, NPAIR], fp32, kind="ExternalInput").ap()
    caug_d = nc.dram_tensor("caug", [128, 8], bf16, kind="ExternalInput").ap()
    scales_d = nc.dram_tensor("scales", [1, 2], fp32, kind="ExternalInput").ap()

    outA_d = nc.dram_tensor("outA", [C, SH, W], fp32, kind="ExternalOutput").ap()
    outM_d = nc.dram_tensor("outM", [SH, W], fp32, kind="ExternalOutput").ap()

    with tile.TileContext(nc) as tc:
        with (
            tc.tile_pool(name="const", bufs=1) as cpool,
            tc.tile_pool(name="arg", bufs=3) as apool,
            tc.tile_pool(name="g", bufs=3) as gpool,
            tc.tile_pool(name="stage", bufs=3) as spool,
            tc.tile_pool(name="big", bufs=1) as bpool,
            tc.tile_pool(name="v", bufs=1) as vpool,
            tc.tile_pool(name="oa", bufs=2) as opool,
            tc.tile_pool(name="psum", bufs=4, space="PSUM") as ppool,
            tc.tile_pool(name="dram", bufs=1, space="DRAM") as dpool,
        ):
            U2 = cpool.tile([128, W], fp32, tag="U2")
            nc.sync.dma_start(U2[:], U2_d[:])
            D2 = cpool.tile([128, W], fp32, tag="D2")
            nc.sync.dma_start(D2[:], D2_d[:])
            zyT = cpool.tile([128, NPAIR], fp32, tag="zyT")
            nc.sync.dma_start(zyT[:], zyT_d[:])
            biasT = cpool.tile([128, NPAIR], fp32, tag="biasT")
            nc.sync.dma_start(biasT[:], biasT_d[:])
            caug = cpool.tile([128, 8], bf16, tag="caug")
            nc.sync.dma_start(caug[:], caug_d[:])
            scs = cpool.tile([1, 2], fp32, tag="scs")
            nc.sync.dma_start(scs[:], scales_d[:])

            # warmup collective: primes the ncfw/axon collective path so the
            # real (latency-critical) ones at the end run warm.
            wup = cpool.tile([1, 8], fp32, tag="wup")
            nc.vector.memset(wup[:], 0.0)
            ccw_in = dpool.tile([1, 8], fp32, tag="ccw_in")
            ccw_out = dpool.tile([1, 8], fp32, tag="ccw_out")
            nc.sync.dma_start(ccw_in[:], wup[:])
            nc.gpsimd.collective_compute(
                "AllReduce", ALU.max, replica_groups=RG,
                ins=[ccw_in.opt()], outs=[ccw_out.opt()],
            )

            # imageN prefetch (used only in pass 2): SBUF layout [h, c, w]
            img = bpool.tile([128, C, W], fp32, tag="img")
            nc.sync.dma_start(img[:], img_d.rearrange("c h w -> h c w"))

            # pass-1 accumulator: [h, ch, w] with ch = (s, n0, n1, n2)
            out4 = bpool.tile([128, 4, W], fp32, tag="out4")

            ones8 = cpool.tile([1, 8], fp32, tag="ones8")
            nc.vector.memset(ones8[:], 1.0)
            pmaxt = cpool.tile([128, 1], fp32, tag="pmaxt")

            cc_in = [None, None]
            cc_out = [None, None]
            for half in range(2):
                cc_in[half] = dpool.tile(
                    [1, 8], fp32, tag=f"cc_in{half}", name=f"cc_in{half}"
                )
                cc_out[half] = dpool.tile(
                    [1, 8], fp32, tag=f"cc_out{half}", name=f"cc_out{half}"
                )

            # ---------------- pass 1 ----------------
            for g in range(NGRP):
                arg = apool.tile([128, 4 * W], fp32, tag="arg")
                for jj in range(4):
                    j = 4 * g + jj
                    nc.vector.affine_then_add(
                        arg[:, jj * W : (jj + 1) * W], U2[:], D2[:],
                        scale=zyT[:, j : j + 1], bias=biasT[:, j : j + 1],
                    )
                G = gpool.tile([128, 4 * W], bf16, tag="G")
                nc.scalar.activation(G[:], arg[:], AF.Exp)
                Pg = ppool.tile([128, W], fp32, tag="P")
                for jj in range(4):
                    p0 = 32 * jj
                    w0 = jj * W
                    nc.tensor.matmul(
                        Pg[p0 : p0 + 8, 0:512], caug[:], G[:, w0 : w0 + 512],
                        tile_position=(0, p0),
                    )
                    nc.tensor.matmul(
                        Pg[p0 : p0 + 8, 512:1024], caug[:],
                        G[:, w0 + 512 : w0 + 1024], tile_position=(0, p0),
                    )
                Sg = spool.tile([128, W], fp32, tag="S")
                nc.scalar.activation(Sg[:], Pg[:], AF.Copy)
                for jj in range(4):
                    h0 = 8 * g + 2 * jj
                    nc.sync.dma_start(
                        out4[h0 : h0 + 2, :, :], Sg[32 * jj : 32 * jj + 8, :]
                    )

                # after each half of the rows: local max -> pipelined
                # AllReduce(max); the first one hides under pass-1 compute.
                if g == NGRP // 2 - 1 or g == NGRP - 1:
                    half = 0 if g == NGRP // 2 - 1 else 1
                    r0 = half * 64
                    nc.vector.reduce_max(
                        pmaxt[r0 : r0 + 64, :], out4[r0 : r0 + 64, 0, :], axis=AX.X
                    )
                    prow = cpool.tile([1, 64], fp32, tag=f"prow{half}")
                    nc.sync.dma_start(prow[:], pmaxt[r0 : r0 + 64, :])
                    mloc = cpool.tile([1, 1], fp32, tag=f"mloc{half}")
                    nc.vector.reduce_max(mloc[:], prow[:], axis=AX.X)
                    m8 = cpool.tile([1, 8], fp32, tag=f"m8{half}")
                    nc.vector.tensor_scalar_mul(m8[:], ones8[:], mloc[:])
                    nc.sync.dma_start(cc_in[half][:], m8[:])
                    nc.gpsimd.collective_compute(
                        "AllReduce", ALU.max, replica_groups=RG,
                        ins=[cc_in[half].opt()], outs=[cc_out[half].opt()],
                    )

            # V_c = imageN_c * s - nr_c (independent of gmax: overlaps the
            # second collective). Post-collective: imageA_c = imageN_c - inv*V_c.
            V = []
            for c in range(C):
                vt = vpool.tile([128, W], fp32, tag=f"V{c}")
                nc.vector.tensor_mul(vt[:], img[:, c, :], out4[:, 0, :])
                nc.vector.tensor_sub(vt[:], vt[:], out4[:, 1 + c, :])
                V.append(vt)

            # inv = +/- scale/gmax on all 128 partitions, via DMA
            # partition-broadcast of each half's collective result.
            mh = []
            for half in range(2):
                mt = cpool.tile([128, 1], fp32, tag=f"mh{half}")
                nc.sync.dma_start(
                    mt[:], cc_out[half][0:1, 0:1].partition_broadcast(128)
                )
                mh.append(mt)
            m128 = cpool.tile([128, 1], fp32, tag="m128")
            nc.vector.tensor_max(m128[:], mh[0][:], mh[1][:])
            rec = cpool.tile([128, 1], fp32, tag="rec")
            nc.vector.reciprocal(rec[:], m128[:])
            scb = cpool.tile([128, 2], fp32, tag="scb")
            nc.sync.dma_start(scb[:], scales_d.partition_broadcast(128))
            inv2 = cpool.tile([128, 2], fp32, tag="inv2")
            nc.vector.tensor_scalar_mul(inv2[:], scb[:], rec[:])

            # ---------------- pass 2 ----------------
            an = opool.tile([128, W], fp32, tag="an")
            nc.vector.tensor_scalar_mul(an[:], out4[:, 0, :], inv2[:, 0:1])
            nc.sync.dma_start(outM_d[:], an[:])
            for c in range(C):
                oA = opool.tile([128, W], fp32, tag="oA")
                # (V_c * -inv) + imageN_c
                nc.vector.scalar_tensor_tensor(
                    oA[:],
                    in0=V[c][:],
                    scalar=inv2[:, 1:2],
                    in1=img[:, c, :],
                    op0=ALU.mult,
                    op1=ALU.add,
                )
                nc.sync.dma_start(outA_d[c], oA[:])

    nc.finalize()
    return nc


def _precompute(color_params, mu_x_params, mu_y_params, sigma_x_params,
                sigma_y_params, rho_params, pi_params, scale_params):
    """CPU parameter transforms (O(K*(H+W)) work) -> per-core input arrays."""
    import ml_dtypes

    f64 = np.float64
    cp = color_params.reshape(C, K).astype(f64)
    mux = np.tanh(mu_x_params.reshape(K).astype(f64))
    muy = np.tanh(mu_y_params.reshape(K).astype(f64))
    sx = np.logaddexp(0.0, sigma_x_params.reshape(K).astype(f64)) + 0.01
    sy = np.logaddexp(0.0, sigma_y_params.reshape(K).astype(f64)) + 0.01
    rho = np.tanh(rho_params.reshape(K).astype(f64)) * 0.99
    pi_p = pi_params.reshape(K).astype(f64)
    lnpi = pi_p - (np.log(np.sum(np.exp(pi_p - pi_p.max()))) + pi_p.max())
    scale = 1.0 / (1.0 + np.exp(-float(scale_params.reshape(()))))
    color = np.tanh(cp)

    q = 1.0 / (2.0 * (1.0 - rho * rho))  # [K]

    # mimic the reference's f32 grid
    x = (np.arange(W, dtype=np.float32) / (W - 1) * 2.0 - 1.0).astype(f64)
    y = (np.arange(H, dtype=np.float32) / (H - 1) * 2.0 - 1.0).astype(f64)

    zx = (x[None, :] - mux[:, None]) / sx[:, None]  # [K, W]
    U = 2.0 * (q * rho)[:, None] * zx  # [K, W]
    D = -q[:, None] * zx * zx  # [K, W]
    zy = (y[None, :] - muy[:, None]) / sy[:, None]  # [K, H]
    e = -q[:, None] * zy * zy + lnpi[:, None]  # [K, H]

    U2 = np.concatenate([U, U], axis=0).astype(np.float32)  # [128, W]
    D2 = np.concatenate([D, D], axis=0).astype(np.float32)

    caug = np.zeros((128, 8), np.float32)
    caug[0:K, 0] = 1.0
    caug[K:128, 4] = 1.0
    for c in range(C):
        caug[0:K, 1 + c] = color[c]
        caug[K:128, 5 + c] = color[c]
    caug = caug.astype(ml_dtypes.bfloat16)

    scales = np.array([[scale, -scale]], np.float32)

    zyTs, biasTs = [], []
    for i in range(NCORES):
        zc = zy[:, i * SH : (i + 1) * SH]  # [K, 128]
        ec = e[:, i * SH : (i + 1) * SH]
        zyTs.append(
            np.concatenate([zc[:, 0::2], zc[:, 1::2]], axis=0).astype(np.float32)
        )
        biasTs.append(
            np.concatenate([ec[:, 0::2], ec[:, 1::2]], axis=0).astype(np.float32)
        )
    return U2, D2, caug, scales, zyTs, biasTs


def _run(inputs, trace=False, trace_cores=None):
    from concourse.bass_utils import run_bass_kernel_spmd

    if "nc" not in _CACHE:
        _CACHE["nc"] = _build_nc()
    nc = _CACHE["nc"]

    imageN = np.ascontiguousarray(np.asarray(inputs["imageN"], np.float32))
    U2, D2, caug, scales, zyTs, biasTs = _precompute(
        inputs["color_params"], inputs["mu_x_params"], inputs["mu_y_params"],
        inputs["sigma_x_params"], inputs["sigma_y_params"], inputs["rho_params"],
        inputs["pi_params"], inputs["scale_params"],
    )

    in_maps = []
    for i in range(NCORES):
        in_maps.append(
            {
                "imgN": np.ascontiguousarray(imageN[0, :, i * SH : (i + 1) * SH, :]),
                "U2": U2,
                "D2": D2,
                "zyT": zyTs[i],
                "biasT": biasTs[i],
                "caug": caug,
                "scales": scales,
            }
        )

    kw = {}
    if trace_cores is not None:
        kw["trace_cores"] = trace_cores
    res = run_bass_kernel_spmd(
        nc, in_maps, core_ids=list(range(NCORES)), trace=trace, **kw
    )

    imageA = np.empty((1, C, H, W), np.float32)
    anomaly = np.empty((1, 1, H, W), np.float32)
    for i in range(NCORES):
        imageA[0, :, i * SH : (i + 1) * SH, :] = res.results[i]["outA"]
        anomaly[0, 0, i * SH : (i + 1) * SH, :] = res.results[i]["outM"]
    return (imageA, anomaly), res


def kernel(**inputs):
    out, _ = _run(inputs, trace=False)
    return out
